# revision 3
# baseline (speedup 1.0000x reference)
"""Trainium2 Bass kernel for nn_GroupGlobalAttention.

Reference math (T=32, B=16, S=128, D=1024):
    pi   = inputs @ W_in.T + b_in                  # [T,B,D]
    pc   = context @ W_ctx.T + b_ctx               # [B,S,D]
    comb = swish(pi[:,:,None,:] + pc[None])        # [T,B,S,D]  (never materialized off-chip)
    scores = comb @ w_one + b_one                  # [T,B,S]
    attn = softmax(scores, -1)                     # [T,B,S]
    attn_context = einsum('tbs,bsd->tbd', attn, context)
Returns (attn_context, attn).

Sharding: data-parallel over B across 8 cores (2 batches/core), weights
replicated.  b_one is ignored: softmax is shift-invariant so it never
affects either output.
"""

import numpy as np
import ml_dtypes

import concourse.bacc as bacc
import concourse.mybir as mybir
from concourse.bass import MemorySpace
from concourse.tile import TileContext
from concourse.bass_utils import run_bass_kernel_spmd

T, B, S, D = 32, 16, 128, 1024
NCORES = 8
BPC = B // NCORES          # batches per core = 2
DC = D // 128              # 8 contraction chunks
EC = D // 128              # 8 output-dim chunks
TS = T * S                 # 4096 (t,s) pairs per batch

F32 = mybir.dt.float32
BF16 = mybir.dt.bfloat16
AF = mybir.ActivationFunctionType
ALU = mybir.AluOpType
AX = mybir.AxisListType

_cached = None


def _build():
    nc = bacc.Bacc()

    d_WcT = nc.dram_tensor("wct", [D, D], BF16, kind="ExternalInput")
    d_WiT = nc.dram_tensor("wit", [D, D], BF16, kind="ExternalInput")
    d_ctxT = nc.dram_tensor("ctxt", [BPC, D, S], BF16, kind="ExternalInput")
    d_ctx = nc.dram_tensor("ctx", [BPC, S, D], F32, kind="ExternalInput")
    d_inpT2 = nc.dram_tensor("inpt2", [D, BPC * T * 2], BF16, kind="ExternalInput")
    d_bias = nc.dram_tensor("bias", [128, EC], F32, kind="ExternalInput")
    d_w1 = nc.dram_tensor("w1", [128, EC], BF16, kind="ExternalInput")
    d_id = nc.dram_tensor("ident", [T, T], F32, kind="ExternalInput")
    d_oc = nc.dram_tensor("out_ctx", [BPC * T, D], F32, kind="ExternalOutput")
    d_oa = nc.dram_tensor("out_attn", [BPC * T, S], F32, kind="ExternalOutput")

    with TileContext(nc) as tc:
        with (
            tc.tile_pool(name="consts", bufs=1) as cpool,
            tc.tile_pool(name="work", bufs=2) as wpool,
            tc.tile_pool(name="psum", bufs=2, space=MemorySpace.PSUM) as ppool,
            tc.tile_pool(name="dram", bufs=2, space=MemorySpace.DRAM) as dpool,
        ):
            # ---- constant loads ----
            ctxT_sb = cpool.tile([128, BPC, DC, S], BF16)
            for b in range(BPC):
                nc.sync.dma_start(
                    ctxT_sb[:, b],
                    d_ctxT[b].rearrange("(dc dp) s -> dp dc s", dp=128),
                )
            WcT_sb = cpool.tile([128, DC, D], BF16)
            WiT_sb = cpool.tile([128, DC, D], BF16)
            for dc in range(DC):
                nc.sync.dma_start(
                    WcT_sb[:, dc], d_WcT[dc * 128 : (dc + 1) * 128, :]
                )
            inpT2_sb = cpool.tile([128, DC, BPC * T * 2], BF16)
            nc.sync.dma_start(
                inpT2_sb, d_inpT2.rearrange("(dc dp) m -> dp dc m", dp=128)
            )
            for dc in range(DC):
                nc.sync.dma_start(
                    WiT_sb[:, dc], d_WiT[dc * 128 : (dc + 1) * 128, :]
                )
            bias_sb = cpool.tile([128, EC], F32)
            nc.sync.dma_start(bias_sb, d_bias[:, :])
            w_sb = cpool.tile([128, EC], BF16)
            nc.sync.dma_start(w_sb, d_w1[:, :])
            id_sb = cpool.tile([T, T], F32)
            nc.sync.dma_start(id_sb, d_id[:, :])
            ctx_sb = cpool.tile([128, BPC, D], F32)
            for b in range(BPC):
                nc.sync.dma_start(ctx_sb[:, b], d_ctx[b])

            # ---- piT2[e, bpc*t*2]: lhsT = WiT chunk, rhs = inpT2 ----
            piT2 = cpool.tile([128, EC, BPC * T * 2], BF16)
            for c in range(EC):
                ps = ppool.tile([128, 128], F32, tag="mm")
                for dc in range(DC):
                    nc.tensor.matmul(
                        ps,
                        WiT_sb[:, dc, c * 128 : (c + 1) * 128],
                        inpT2_sb[:, dc],
                        start=(dc == 0),
                        stop=(dc == DC - 1),
                    )
                nc.vector.tensor_copy(piT2[:, c], ps)

            # ---- per (b, c): pcT matmul, comb broadcast-add, swish ----
            swishes = [[None] * EC for _ in range(BPC)]
            for b in range(BPC):
                for c in range(EC):
                    ps = ppool.tile([128, 128], F32, tag="mm")
                    for dc in range(DC):
                        nc.tensor.matmul(
                            ps,
                            WcT_sb[:, dc, c * 128 : (c + 1) * 128],
                            ctxT_sb[:, b, dc],
                            start=(dc == 0),
                            stop=(dc == DC - 1),
                        )
                    pcT = wpool.tile([128, S], BF16, tag="pcT", bufs=4)
                    nc.vector.tensor_copy(pcT, ps)

                    comb = wpool.tile([128, TS], BF16, tag="comb", bufs=3)
                    in0 = (
                        pcT.rearrange("p (r j) -> p r j", j=2)[:, None, :, :]
                        .broadcast_to([128, T, S // 2, 2])
                    )
                    in1 = (
                        piT2[:, c, b * (T * 2) : (b + 1) * (T * 2)]
                        .rearrange("p (t j) -> p t j", j=2)[:, :, None, :]
                        .broadcast_to([128, T, S // 2, 2])
                    )
                    nc.vector.tensor_add(
                        comb.rearrange("p (t r j) -> p t r j", t=T, j=2), in0, in1
                    )
                    swish = wpool.tile([128, TS], BF16, tag="swish", bufs=10)
                    nc.scalar.activation(
                        swish, comb, AF.Silu, bias=bias_sb[:, c : c + 1], scale=1.0
                    )
                    swishes[b][c] = swish

            # ---- scores: psum[1, 512] accumulation over chunks ----
            srows = []
            for b in range(BPC):
                srow = wpool.tile([1, TS], F32, tag="srow", bufs=1)
                for j in range(TS // 512):
                    ps = ppool.tile([1, 512], F32, tag="sc")
                    for c in range(EC):
                        nc.tensor.matmul(
                            ps,
                            w_sb[:, c : c + 1],
                            swishes[b][c][:, j * 512 : (j + 1) * 512],
                            start=(c == 0),
                            stop=(c == EC - 1),
                        )
                    nc.vector.tensor_copy(srow[:, j * 512 : (j + 1) * 512], ps)
                srows.append(srow)

            # ---- softmax + attn@context ----
            for b in range(BPC):
                scr = dpool.tile([T, S], F32, tag="scr")
                nc.sync.dma_start(scr, srows[b])
                ssb = wpool.tile([T, S], F32, tag="ssb")
                nc.sync.dma_start(ssb, scr)
                negmax = wpool.tile([T, 1], F32, tag="ngm")
                nc.vector.tensor_reduce(
                    negmax, ssb, axis=AX.X, op=ALU.max, negate=True
                )
                expt = wpool.tile([T, S], F32, tag="expt")
                zsum = wpool.tile([T, 1], F32, tag="zs")
                nc.scalar.activation(
                    expt, ssb, AF.Exp, bias=negmax[:, 0:1], scale=1.0,
                    accum_out=zsum[:, 0:1],
                )
                rz = wpool.tile([T, 1], F32, tag="rz")
                nc.vector.reciprocal(rz, zsum)
                attn_sb = wpool.tile([T, S], F32, tag="attn")
                nc.vector.tensor_scalar_mul(attn_sb, expt, rz[:, 0:1])
                nc.sync.dma_start(d_oa[b * T : (b + 1) * T, :], attn_sb)

                psT = ppool.tile([S, T], F32, tag="at")
                nc.tensor.transpose(psT, attn_sb, id_sb)
                attnT = wpool.tile([S, T], F32, tag="attnT")
                nc.vector.tensor_copy(attnT, psT)
                oc_sb = wpool.tile([T, D], F32, tag="oc")
                for h in range(2):
                    pac = ppool.tile([T, 512], F32, tag="ac")
                    nc.tensor.matmul(
                        pac, attnT, ctx_sb[:, b, h * 512 : (h + 1) * 512],
                        start=True, stop=True,
                    )
                    nc.vector.tensor_copy(oc_sb[:, h * 512 : (h + 1) * 512], pac)
                nc.sync.dma_start(d_oc[b * T : (b + 1) * T, :], oc_sb)

    nc.compile()
    return nc


def _prep_inputs(inputs, context, W_in, b_in, W_ctx, b_ctx, w_one):
    bf = ml_dtypes.bfloat16
    WcT = np.ascontiguousarray(W_ctx.T).astype(bf)
    WiT = np.ascontiguousarray(W_in.T).astype(bf)
    bias = np.ascontiguousarray((b_in + b_ctx).reshape(EC, 128).T).astype(np.float32)
    w1 = np.ascontiguousarray(w_one.reshape(EC, 128).T).astype(bf)
    ident = np.eye(T, dtype=np.float32)

    in_maps = []
    for k in range(NCORES):
        bsl = slice(BPC * k, BPC * (k + 1))
        ctx_k = np.ascontiguousarray(context[bsl]).astype(np.float32)
        ctxT_k = np.ascontiguousarray(context[bsl].transpose(0, 2, 1)).astype(bf)
        m = np.ascontiguousarray(inputs[:, bsl, :].transpose(1, 0, 2)).reshape(
            BPC * T, D
        )
        inpT2_k = np.ascontiguousarray(np.repeat(m, 2, axis=0).T).astype(bf)
        in_maps.append(
            {
                "wct": WcT,
                "wit": WiT,
                "ctxt": ctxT_k,
                "ctx": ctx_k,
                "inpt2": inpT2_k,
                "bias": bias,
                "w1": w1,
                "ident": ident,
            }
        )
    return in_maps


def kernel(inputs, context, W_in, b_in, W_ctx, b_ctx, w_one, b_one):
    global _cached
    inputs = np.asarray(inputs, np.float32)
    context = np.asarray(context, np.float32)
    W_in = np.asarray(W_in, np.float32)
    b_in = np.asarray(b_in, np.float32)
    W_ctx = np.asarray(W_ctx, np.float32)
    b_ctx = np.asarray(b_ctx, np.float32)
    w_one = np.asarray(w_one, np.float32)

    if _cached is None:
        _cached = _build()
    nc = _cached

    in_maps = _prep_inputs(inputs, context, W_in, b_in, W_ctx, b_ctx, w_one)
    res = run_bass_kernel_spmd(nc, in_maps, core_ids=list(range(NCORES)))

    attn_ctx = np.empty((T, B, D), np.float32)
    attn = np.empty((T, B, S), np.float32)
    for k in range(NCORES):
        bsl = slice(BPC * k, BPC * (k + 1))
        attn_ctx[:, bsl, :] = (
            res.results[k]["out_ctx"].reshape(BPC, T, D).transpose(1, 0, 2)
        )
        attn[:, bsl, :] = (
            res.results[k]["out_attn"].reshape(BPC, T, S).transpose(1, 0, 2)
        )
    return attn_ctx, attn
